# revision 16
# baseline (speedup 1.0000x reference)
"""Trainium2 Bass kernel for BinaryTreeLatentVariable inside algorithm.

Math (per level d, bottom-up over a complete binary tree in heap order):
    new[pp, n] = p[pp, n] + logsumexp_{i,j}( trans[pp, i, j] + l[i, n] + r[j, n] )

Factorization (s[n] = l[0, n] + r[0, n]):
    new[pp, n] = p[pp, n] + s[n] + log( sum_{ij} expT[ij, pp] * V[ij, n] )
    V[ij, n]   = exp( lnorm[i, n] + rnorm[j, n] ),  xnorm[i] = x[i] - x[0]
    expT       = exp(trans) permuted to [(lL,lc),(rL,rc)] x [(pL,pc)]

Representation: levels are stored NORMALIZED (relative to each node's
state-0 score, so values stay within ~+-12 and bf16 matmuls are safe) and
DEINTERLEAVED (sibling pairs share a column: left child in partitions
0..19, right child in partitions 64..83; state-0 rows carry don't-care
absolute values — the select matrices have zero weight there because
xnorm[0] == 0).  Absolute state-0 scores flow through a separate fp32
z-chain: zsum[n] = l[0,n] + r[0,n] per parent node.

Per level tile (nodes on the free axis):
    - 4x select matmul (K=84, bf16 0/1 matrix): args = lnorm_i + rnorm_j,
      written into paired PSUM banks so one ACT exp covers two chunks
    - 2x ACT exp (PSUM pair -> SBUF bf16), 4x contraction matmul with expT
    - ACT ln of the accumulated sums, DVE u = ln + p, normalization matmul
      (K=20, columns e_i - e_0; column 0 = e_0 keeps the absolute row),
      DVE deinterleave into the next ybuf
    - GPSIMD carries the fp32 z-chain (zrow = ln0 + (p0 + zsum); pairs)

Phase 1: emission sw = W^T @ hT + b on PE; h is cast to bf16 host-side
(halving HBM traffic) and laid out level-major (leaves first) so the
deepest level overlaps the tail of the h DMA.  Leaf columns use
host-normalized weights Wn (column i -> W_i - W_0, except column 0) so
leaf outputs drop straight into the normalized representation.

Sharding: 8 trees per core across 8 cores (no cross-core communication).
"""

import ml_dtypes
import numpy as np

import concourse.bacc as bacc
import concourse.bass as bass
from concourse import mybir, tile
from concourse.bass_utils import run_bass_kernel_spmd

F32 = mybir.dt.float32
BF16 = mybir.dt.bfloat16
NP_BF16 = ml_dtypes.bfloat16

B = 64
N_NODES = 1023
D = 512
L = 5
C = 4
LC = L * C          # 20
IJ = 400            # 20 * 20
NCORES = 8
TPC = B // NCORES   # trees per core = 8
DEPTH = 9           # leaves are level 9; internal levels 8..0

# Per-core column layout: level-major blocks (leaves first), t-major inside.
LEVEL_ORDER = list(range(DEPTH, -1, -1))  # 9, 8, ..., 0
OFFS = {}
_off = 0
for _d in LEVEL_ORDER:
    OFFS[_d] = _off
    _off += TPC * (1 << _d)
NCOL = _off                      # 8184
NLEAFC = TPC * (1 << DEPTH)      # 4096 leaf columns
NCOLI = NCOL - NLEAFC            # 4088 internal columns
OFFSI = {d: OFFS[d] - NLEAFC for d in range(DEPTH)}

COLTILE = 512
# DMA column groups: small first tiles so the first matmul starts early,
# then large tiles to amortize descriptor issue cost. Sums to NCOL=8184.
DMA_GROUPS = [512, 512, 1024, 2048, 2048, 2040]
ROWR = 64           # partition base of the right-child (odd) block
NROWY = 84          # ybuf partitions: 0..19 left, 64..83 right, rest zero
KCH = 4             # 400 = 4 x 100 chunks of the ij axis
CHW = IJ // KCH     # 100


def _host_constants(W, b, trans):
    # expT: [400, 20], row = (lL*4+lc)*20 + (rL*4+rc), col = pL*4+pc,
    # chunked to [100, 4, 20] so SBUF tiles slice on a free dim.
    expT = np.exp(trans.astype(np.float64).transpose(1, 4, 2, 5, 0, 3)
                  .reshape(IJ, LC))
    expT_ch = np.ascontiguousarray(
        expT.reshape(KCH, CHW, LC).transpose(1, 0, 2)).astype(NP_BF16)

    ij = np.arange(IJ)
    selLR = np.zeros((NROWY, IJ), NP_BF16)
    selLR[ij // LC, ij] = 1.0            # left-child state select
    selLR[ROWR + ij % LC, ij] = 1.0      # right-child state select
    selLR[0, :] = 0.0                    # lnorm[0] == 0: ignore row 0
    selLR[ROWR, :] = 0.0                 # rnorm[0] == 0: ignore row 64

    # normalized weights everywhere: col i -> W_i - W_0 for i>0; col 0
    # keeps W_0 so row 0 of every emission is the absolute state-0 score.
    Wn = W - W[:, 0:1]
    Wn[:, 0] = W[:, 0]
    w_both = np.ascontiguousarray(
        Wn.reshape(KCH, D // KCH, LC)).astype(NP_BF16)

    bn = (b - b[0]).astype(np.float32)
    bn[0] = b[0]
    bcols = bn.reshape(LC, 1)

    # normalization: col i>0 -> u_i - u_0; col 0 -> u_0 (absolute kept)
    normmat = np.zeros((LC, LC), NP_BF16)
    normmat[0, 0] = 1.0
    for i in range(1, LC):
        normmat[i, i] = 1.0
        normmat[0, i] = -1.0
    ones_row = np.ones((1, LC), np.float32)
    return {
        "expt": expT_ch, "sellr": selLR, "wboth": w_both, "bcols": bcols,
        "normmat": normmat, "onesr": ones_row,
    }


def _host_ht(h, core):
    """bf16 [512, NCOL] slice for one core: level-major, t-major inside."""
    hk = h[core * TPC:(core + 1) * TPC]          # [8, 1023, 512]
    blocks = []
    for d in LEVEL_ORDER:
        lo, hi = (1 << d) - 1, (1 << (d + 1)) - 1
        blk = hk[:, lo:hi, :]                     # [8, m, 512]
        blocks.append(blk.transpose(2, 0, 1).reshape(D, -1))
    out = np.concatenate(blocks, axis=1)
    return np.ascontiguousarray(out).astype(NP_BF16)


def _patch_act_tables(nc):
    """Retarget every activation-table load to natural_log_exp_and_others
    (covers Exp, Ln and Identity) and drop the now-redundant reloads, which
    otherwise cost ~1.3us each when Exp and Ln alternate."""
    from concourse.hw_specs import get_activation_tables
    tables = list(get_activation_tables(nc.m.arch).items())
    target = None
    for idx, (name, _fns) in enumerate(tables):
        if name == "natural_log_exp_and_others":
            target = idx
    if target is None:
        return
    for fn in nc.m.functions:
        kept = False
        for blk in fn.blocks:
            new_insts = []
            for ins in blk.instructions:
                if isinstance(ins, mybir.InstLoadActFuncSet):
                    si = ins.sync_info
                    has_sems = si is not None and (
                        len(si.on_wait) > 0 or len(si.on_update) > 0)
                    if not kept or has_sems:
                        ins.act_func_set_id = target
                        kept = True
                        new_insts.append(ins)
                    continue
                new_insts.append(ins)
            blk.instructions[:] = new_insts


def _build_bass():
    nc = bacc.Bacc("TRN2", target_bir_lowering=False)

    ht_d = nc.declare_dram_parameter("ht", [D, NCOL], BF16, isOutput=False)
    wboth_d = nc.declare_dram_parameter("wboth", [KCH, D // KCH, LC],
                                        BF16, isOutput=False)
    bcols_d = nc.declare_dram_parameter("bcols", [LC, 1], F32, isOutput=False)
    expt_d = nc.declare_dram_parameter("expt", [CHW, KCH, LC], BF16,
                                       isOutput=False)
    sellr_d = nc.declare_dram_parameter("sellr", [NROWY, IJ], BF16,
                                        isOutput=False)
    normmat_d = nc.declare_dram_parameter("normmat", [LC, LC], BF16,
                                          isOutput=False)
    onesr_d = nc.declare_dram_parameter("onesr", [1, LC], F32, isOutput=False)
    out_d = nc.declare_dram_parameter("out", [LC, TPC], F32, isOutput=True)

    EXP = mybir.ActivationFunctionType.Exp
    LN = mybir.ActivationFunctionType.Ln
    ADD = mybir.AluOpType.add

    with tile.TileContext(nc) as tc:
        with (
            tc.tile_pool(name="consts", bufs=1) as consts,
            tc.tile_pool(name="sw", bufs=1) as swp,
            tc.tile_pool(name="ybufs", bufs=1) as ybp,
            tc.tile_pool(name="ht", bufs=8) as htp,
            tc.tile_pool(name="vtiles", bufs=4) as vtp,
            tc.tile_pool(name="utiles", bufs=3) as utp,
            tc.tile_pool(name="ttiles", bufs=2) as ttp,
            tc.tile_pool(name="ps_sw", bufs=1, space="PSUM") as ps_swp,
            tc.tile_pool(name="ps_exp", bufs=2, space="PSUM") as ps_expp,
            tc.tile_pool(name="ps_out", bufs=2, space="PSUM") as ps_outp,
            tc.tile_pool(name="ps_norm", bufs=1, space="PSUM") as ps_normp,
        ):
            # ---- constants (issued on the GPSIMD queue so the SP queue
            # starts streaming hT immediately) ----
            w_sb = consts.tile([D // KCH, KCH, LC], BF16)
            nc.gpsimd.dma_start(w_sb[:], wboth_d[:].transpose([1, 0, 2]))
            expt_sb = consts.tile([CHW, KCH, LC], BF16)
            nc.gpsimd.dma_start(expt_sb[:], expt_d[:])
            sellr_sb = consts.tile([NROWY, IJ], BF16)
            nc.gpsimd.dma_start(sellr_sb[:], sellr_d[:])
            b_sb = consts.tile([LC, 1], F32)
            nc.gpsimd.dma_start(b_sb[:], bcols_d[:])
            normmat_sb = consts.tile([LC, LC], BF16)
            nc.gpsimd.dma_start(normmat_sb[:], normmat_d[:])
            onesr_sb = consts.tile([1, LC], F32)
            nc.gpsimd.dma_start(onesr_sb[:], onesr_d[:])

            # normalized emissions of internal nodes (row 0 = absolute p0)
            sw_sb = swp.tile([LC, NCOLI], F32)
            # per-tree accumulator of absolute state-0 scores (z-total)
            zacc = swp.tile([1, TPC], F32)
            zfin = swp.tile([1, TPC], F32)
            nc.vector.memset(zacc[:], 0.0)

            # per-level normalized deinterleaved buffers, bf16
            ybufs = {}
            for d in range(DEPTH, 0, -1):
                npair = TPC * (1 << d) // 2
                yb = ybp.tile([NROWY, npair], BF16, tag=f"y{d}", name=f"y{d}")
                nc.gpsimd.memset(yb[:], 0.0)
                ybufs[d] = yb

            # ---- phase 1: sw / leaf level from hT ----
            ct = 0
            for dw in DMA_GROUPS:
                htts = []
                for kd in range(KCH):
                    htt = htp.tile([D // KCH, dw], BF16, tag=f"htt{dw}",
                                   name="htt")
                    nc.sync.dma_start(
                        htt[:, :dw],
                        ht_d[kd * (D // KCH):(kd + 1) * (D // KCH),
                             ct:ct + dw])
                    htts.append(htt)
                for half in range(0, dw, COLTILE):
                    nt = min(COLTILE, dw - half)
                    c0 = ct + half
                    leaf = c0 < NLEAFC
                    ps = ps_swp.tile([LC, COLTILE], F32, tag="ps_sw",
                                     name="ps_sw")
                    for kd in range(KCH):
                        nc.tensor.matmul(
                            ps[:, :nt], w_sb[:, kd, :],
                            htts[kd][:, half:half + nt],
                            start=(kd == 0), stop=(kd == KCH - 1))
                    if leaf:
                        # normalized already; deinterleave + bias into y9
                        pair0 = c0 // 2
                        nh = nt // 2
                        y9 = ybufs[DEPTH]
                        nc.vector.tensor_scalar(
                            y9[0:LC, pair0:pair0 + nh], ps[:, 0:nt:2],
                            b_sb[:, 0:1], None, ADD)
                        nc.vector.tensor_scalar(
                            y9[ROWR:ROWR + LC, pair0:pair0 + nh],
                            ps[:, 1:nt:2], b_sb[:, 0:1], None, ADD)
                    else:
                        nc.vector.tensor_scalar(
                            sw_sb[0:LC, c0 - NLEAFC:c0 - NLEAFC + nt],
                            ps[:, :nt], b_sb[:, 0:1], None, ADD)
                ct += dw

            # ---- phase 2: bottom-up tree levels ----
            for d in range(DEPTH - 1, -1, -1):
                n = TPC * (1 << d)
                yprev = ybufs[d + 1]
                p_off = OFFSI[d]
                # fold the completed child level's absolute state-0 scores
                # (ybuf rows 0 and 64) into the per-tree z accumulator
                npair_pt = (1 << d)          # pairs per tree in yprev
                for row in (0, ROWR):
                    rsum = utp.tile([1, TPC], F32, tag="rsum", name="rsum")
                    nc.vector.tensor_reduce(
                        rsum[:], yprev[row:row + 1, :].rearrange(
                            "p (t q) -> p t q", t=TPC),
                        mybir.AxisListType.X, ADD)
                    nc.vector.tensor_add(zacc[:], zacc[:], rsum[:])
                for c0 in range(0, n, COLTILE):
                    nt = min(COLTILE, n - c0)
                    ops_ = ps_outp.tile([LC, COLTILE], F32, tag="ps_out",
                                        name="ps_out")
                    for kp in range(KCH // 2):
                        eps = ps_expp.tile([CHW, 2, COLTILE], F32,
                                           tag="ps_exp", name="ps_exp")
                        for kk in range(2):
                            kc = 2 * kp + kk
                            nc.tensor.matmul(
                                eps[:, kk, :nt],
                                sellr_sb[:, kc * CHW:(kc + 1) * CHW],
                                yprev[0:NROWY, c0:c0 + nt],
                                start=True, stop=True)
                        v_sb = vtp.tile([CHW, 2, COLTILE], BF16, tag="v",
                                        name="v")
                        nc.scalar.activation(v_sb[:, :, :nt],
                                             eps[:, :, :nt], EXP)
                        for kk in range(2):
                            kc = 2 * kp + kk
                            nc.tensor.matmul(
                                ops_[:, :nt], expt_sb[:, kc, :],
                                v_sb[:, kk, :nt],
                                start=(kc == 0), stop=(kc == KCH - 1))

                    tdt = F32 if d == 0 else BF16
                    t_sb = ttp.tile([LC, COLTILE], tdt, tag=f"t{tdt}",
                                    name="t")
                    nc.scalar.activation(t_sb[:, :nt], ops_[:, :nt], LN)

                    if d == 0:
                        # final: Y = t + p_norm + (ztotal + p0_root) bcast,
                        # then undo the double-counted p0 on row 0.
                        nc.vector.tensor_add(
                            zfin[:], zacc[:],
                            sw_sb[0:1, p_off:p_off + nt])
                        qps = ps_normp.tile([LC, COLTILE], F32,
                                            tag="ps_norm", name="ps_norm")
                        nc.tensor.matmul(qps[:, :nt], onesr_sb[:],
                                         zfin[:], start=True, stop=True)
                        y0a = utp.tile([LC, TPC], F32, tag="y0a", name="y0a")
                        nc.vector.tensor_add(
                            y0a[:], t_sb[:, :nt],
                            sw_sb[0:LC, p_off:p_off + nt])
                        y0b = utp.tile([LC, TPC], F32, tag="y0b", name="y0b")
                        nc.vector.tensor_add(y0b[:], y0a[:], qps[:, :nt])
                        nc.vector.tensor_sub(
                            y0b[0:1, :], y0b[0:1, :],
                            sw_sb[0:1, p_off:p_off + nt])
                        nc.sync.dma_start(out_d[:], y0b[:])
                        continue

                    # normalize ln-scores, add normalized p, deinterleave
                    pn = ps_normp.tile([LC, COLTILE], F32, tag="ps_norm",
                                       name="ps_norm")
                    nc.tensor.matmul(pn[:, :nt], normmat_sb[:],
                                     t_sb[:, :nt], start=True, stop=True)
                    pair0 = c0 // 2
                    nh = nt // 2
                    yb = ybufs[d]
                    nc.vector.tensor_add(
                        yb[0:LC, pair0:pair0 + nh], pn[:, 0:nt:2],
                        sw_sb[0:LC, p_off + c0:p_off + c0 + nt:2])
                    nc.vector.tensor_add(
                        yb[ROWR:ROWR + LC, pair0:pair0 + nh], pn[:, 1:nt:2],
                        sw_sb[0:LC, p_off + c0 + 1:p_off + c0 + nt:2])

    nc.compile()
    _patch_act_tables(nc)
    return nc


_CACHE = {}


def _get_nc():
    if "nc" not in _CACHE:
        _CACHE["nc"] = _build_bass()
    return _CACHE["nc"]


def run(h, W, b, trans, trace=False, **trace_kwargs):
    h = np.asarray(h, dtype=np.float32)
    W = np.asarray(W, dtype=np.float32)
    b = np.asarray(b, dtype=np.float32)
    trans = np.asarray(trans, dtype=np.float32)

    consts = _host_constants(W, b, trans)
    in_maps = []
    for core in range(NCORES):
        m = dict(consts)
        m["ht"] = _host_ht(h, core)
        in_maps.append(m)

    nc = _get_nc()
    res = run_bass_kernel_spmd(nc, in_maps, list(range(NCORES)),
                               trace=trace, **trace_kwargs)
    outs = [res.results[k]["out"] for k in range(NCORES)]  # each [20, 8]
    full = np.concatenate([np.asarray(o, np.float32).T for o in outs],
                          axis=0).reshape(B, L, C)
    return np.ascontiguousarray(full), res


def kernel(h, W, b, trans):
    out, _ = run(h, W, b, trans, trace=False)
    return out


# revision 17
# speedup vs baseline: 1.0257x; 1.0257x over previous
"""Trainium2 Bass kernel for BinaryTreeLatentVariable inside algorithm.

Math (per level d, bottom-up over a complete binary tree in heap order):
    new[pp, n] = p[pp, n] + logsumexp_{i,j}( trans[pp, i, j] + l[i, n] + r[j, n] )

Factorization (s[n] = l[0, n] + r[0, n]):
    new[pp, n] = p[pp, n] + s[n] + log( sum_{ij} expT[ij, pp] * V[ij, n] )
    V[ij, n]   = exp( lnorm[i, n] + rnorm[j, n] ),  xnorm[i] = x[i] - x[0]
    expT       = exp(trans) permuted to [(lL,lc),(rL,rc)] x [(pL,pc)]

Representation: levels are stored NORMALIZED (relative to each node's
state-0 score, so values stay within ~+-12 and bf16 matmuls are safe) and
DEINTERLEAVED (sibling pairs share a column: left child in partitions
0..19, right child in partitions 64..83; state-0 rows carry don't-care
absolute values — the select matrices have zero weight there because
xnorm[0] == 0).  Absolute state-0 scores flow through a separate fp32
z-chain: zsum[n] = l[0,n] + r[0,n] per parent node.

Per level tile (nodes on the free axis):
    - 4x select matmul (K=84, bf16 0/1 matrix): args = lnorm_i + rnorm_j,
      written into paired PSUM banks so one ACT exp covers two chunks
    - 2x ACT exp (PSUM pair -> SBUF bf16), 4x contraction matmul with expT
    - ACT ln of the accumulated sums, DVE u = ln + p, normalization matmul
      (K=20, columns e_i - e_0; column 0 = e_0 keeps the absolute row),
      DVE deinterleave into the next ybuf
    - GPSIMD carries the fp32 z-chain (zrow = ln0 + (p0 + zsum); pairs)

Phase 1: emission sw = W^T @ hT + b on PE; h is cast to bf16 host-side
(halving HBM traffic) and laid out level-major (leaves first) so the
deepest level overlaps the tail of the h DMA.  Leaf columns use
host-normalized weights Wn (column i -> W_i - W_0, except column 0) so
leaf outputs drop straight into the normalized representation.

Sharding: 8 trees per core across 8 cores (no cross-core communication).
"""

import ml_dtypes
import numpy as np

import concourse.bacc as bacc
import concourse.bass as bass
from concourse import mybir, tile
from concourse.bass_utils import run_bass_kernel_spmd

F32 = mybir.dt.float32
BF16 = mybir.dt.bfloat16
NP_BF16 = ml_dtypes.bfloat16

B = 64
N_NODES = 1023
D = 512
L = 5
C = 4
LC = L * C          # 20
IJ = 400            # 20 * 20
NCORES = 8
TPC = B // NCORES   # trees per core = 8
DEPTH = 9           # leaves are level 9; internal levels 8..0

# Per-core column layout: level-major blocks (leaves first), t-major inside.
LEVEL_ORDER = list(range(DEPTH, -1, -1))  # 9, 8, ..., 0
OFFS = {}
_off = 0
for _d in LEVEL_ORDER:
    OFFS[_d] = _off
    _off += TPC * (1 << _d)
NCOL = _off                      # 8184
NLEAFC = TPC * (1 << DEPTH)      # 4096 leaf columns
NCOLI = NCOL - NLEAFC            # 4088 internal columns
OFFSI = {d: OFFS[d] - NLEAFC for d in range(DEPTH)}

COLTILE = 512
# DMA column groups: small first tiles so the first matmul starts early,
# then large tiles to amortize descriptor issue cost. Sums to NCOL=8184.
DMA_GROUPS = [512, 512, 1024, 2048, 2048, 2040]
ROWR = 64           # partition base of the right-child (odd) block
NROWY = 84          # ybuf partitions: 0..19 left, 64..83 right, rest zero
KCH = 4             # 400 = 4 x 100 chunks of the ij axis
CHW = IJ // KCH     # 100


def _host_constants(W, b, trans):
    # expT: [400, 20], row = (lL*4+lc)*20 + (rL*4+rc), col = pL*4+pc,
    # chunked to [100, 4, 20] so SBUF tiles slice on a free dim.
    expT = np.exp(trans.astype(np.float64).transpose(1, 4, 2, 5, 0, 3)
                  .reshape(IJ, LC))
    expT_ch = np.ascontiguousarray(
        expT.reshape(KCH, CHW, LC).transpose(1, 0, 2)).astype(NP_BF16)

    ij = np.arange(IJ)
    selLR = np.zeros((NROWY, IJ), NP_BF16)
    selLR[ij // LC, ij] = 1.0            # left-child state select
    selLR[ROWR + ij % LC, ij] = 1.0      # right-child state select
    selLR[0, :] = 0.0                    # lnorm[0] == 0: ignore row 0
    selLR[ROWR, :] = 0.0                 # rnorm[0] == 0: ignore row 64

    # normalized weights everywhere: col i -> W_i - W_0 for i>0; col 0
    # keeps W_0 so row 0 of every emission is the absolute state-0 score.
    Wn = W - W[:, 0:1]
    Wn[:, 0] = W[:, 0]
    w_both = np.ascontiguousarray(
        Wn.reshape(KCH, D // KCH, LC)).astype(NP_BF16)

    bn = (b - b[0]).astype(np.float32)
    bn[0] = b[0]
    bcols = bn.reshape(LC, 1)

    # normalization: col i>0 -> u_i - u_0; col 0 -> u_0 (absolute kept)
    normmat = np.zeros((LC, LC), NP_BF16)
    normmat[0, 0] = 1.0
    for i in range(1, LC):
        normmat[i, i] = 1.0
        normmat[0, i] = -1.0
    ones_row = np.ones((1, LC), np.float32)
    return {
        "expt": expT_ch, "sellr": selLR, "wboth": w_both, "bcols": bcols,
        "normmat": normmat, "onesr": ones_row,
    }


def _host_ht(h, core):
    """bf16 [512, NCOL] slice for one core: level-major, t-major inside."""
    hk = h[core * TPC:(core + 1) * TPC]          # [8, 1023, 512]
    blocks = []
    for d in LEVEL_ORDER:
        lo, hi = (1 << d) - 1, (1 << (d + 1)) - 1
        blk = hk[:, lo:hi, :]                     # [8, m, 512]
        blocks.append(blk.transpose(2, 0, 1).reshape(D, -1))
    out = np.concatenate(blocks, axis=1)
    return np.ascontiguousarray(out).astype(NP_BF16)


def _patch_act_tables(nc):
    """Retarget every activation-table load to natural_log_exp_and_others
    (covers Exp, Ln and Identity) and drop the now-redundant reloads, which
    otherwise cost ~1.3us each when Exp and Ln alternate."""
    from concourse.hw_specs import get_activation_tables
    tables = list(get_activation_tables(nc.m.arch).items())
    target = None
    for idx, (name, _fns) in enumerate(tables):
        if name == "natural_log_exp_and_others":
            target = idx
    if target is None:
        return
    for fn in nc.m.functions:
        kept = False
        for blk in fn.blocks:
            new_insts = []
            for ins in blk.instructions:
                if isinstance(ins, mybir.InstLoadActFuncSet):
                    si = ins.sync_info
                    has_sems = si is not None and (
                        len(si.on_wait) > 0 or len(si.on_update) > 0)
                    if not kept or has_sems:
                        ins.act_func_set_id = target
                        kept = True
                        new_insts.append(ins)
                    continue
                new_insts.append(ins)
            blk.instructions[:] = new_insts


def _build_bass():
    nc = bacc.Bacc("TRN2", target_bir_lowering=False)

    ht_d = nc.declare_dram_parameter("ht", [D, NCOL], BF16, isOutput=False)
    wboth_d = nc.declare_dram_parameter("wboth", [KCH, D // KCH, LC],
                                        BF16, isOutput=False)
    bcols_d = nc.declare_dram_parameter("bcols", [LC, 1], F32, isOutput=False)
    expt_d = nc.declare_dram_parameter("expt", [CHW, KCH, LC], BF16,
                                       isOutput=False)
    sellr_d = nc.declare_dram_parameter("sellr", [NROWY, IJ], BF16,
                                        isOutput=False)
    normmat_d = nc.declare_dram_parameter("normmat", [LC, LC], BF16,
                                          isOutput=False)
    onesr_d = nc.declare_dram_parameter("onesr", [1, LC], F32, isOutput=False)
    out_d = nc.declare_dram_parameter("out", [LC, TPC], F32, isOutput=True)

    EXP = mybir.ActivationFunctionType.Exp
    LN = mybir.ActivationFunctionType.Ln
    ADD = mybir.AluOpType.add

    with tile.TileContext(nc) as tc:
        with (
            tc.tile_pool(name="consts", bufs=1) as consts,
            tc.tile_pool(name="sw", bufs=1) as swp,
            tc.tile_pool(name="ybufs", bufs=1) as ybp,
            tc.tile_pool(name="ht", bufs=8) as htp,
            tc.tile_pool(name="vtiles", bufs=4) as vtp,
            tc.tile_pool(name="utiles", bufs=3) as utp,
            tc.tile_pool(name="ttiles", bufs=2) as ttp,
            tc.tile_pool(name="ps_sw", bufs=2, space="PSUM") as ps_swp,
            tc.tile_pool(name="ps_exp", bufs=2, space="PSUM") as ps_expp,
            tc.tile_pool(name="ps_out", bufs=1, space="PSUM") as ps_outp,
            tc.tile_pool(name="ps_norm", bufs=1, space="PSUM") as ps_normp,
        ):
            # ---- constants (issued on the GPSIMD queue so the SP queue
            # starts streaming hT immediately) ----
            w_sb = consts.tile([D // KCH, KCH, LC], BF16)
            nc.gpsimd.dma_start(w_sb[:], wboth_d[:].transpose([1, 0, 2]))
            expt_sb = consts.tile([CHW, KCH, LC], BF16)
            nc.gpsimd.dma_start(expt_sb[:], expt_d[:])
            sellr_sb = consts.tile([NROWY, IJ], BF16)
            nc.gpsimd.dma_start(sellr_sb[:], sellr_d[:])
            b_sb = consts.tile([LC, 1], F32)
            nc.gpsimd.dma_start(b_sb[:], bcols_d[:])
            normmat_sb = consts.tile([LC, LC], BF16)
            nc.gpsimd.dma_start(normmat_sb[:], normmat_d[:])
            onesr_sb = consts.tile([1, LC], F32)
            nc.gpsimd.dma_start(onesr_sb[:], onesr_d[:])

            # normalized emissions of internal nodes (row 0 = absolute p0)
            sw_sb = swp.tile([LC, NCOLI], F32)
            # per-tree accumulator of absolute state-0 scores (z-total)
            zacc = swp.tile([1, TPC], F32)
            zfin = swp.tile([1, TPC], F32)
            nc.vector.memset(zacc[:], 0.0)

            # per-level normalized deinterleaved buffers, bf16
            ybufs = {}
            for d in range(DEPTH, 0, -1):
                npair = TPC * (1 << d) // 2
                yb = ybp.tile([NROWY, npair], BF16, tag=f"y{d}", name=f"y{d}")
                nc.gpsimd.memset(yb[:], 0.0)
                ybufs[d] = yb

            # ---- phase 1: sw / leaf level from hT ----
            ct = 0
            for dw in DMA_GROUPS:
                htts = []
                for kd in range(KCH):
                    htt = htp.tile([D // KCH, dw], BF16, tag=f"htt{dw}",
                                   name="htt")
                    nc.sync.dma_start(
                        htt[:, :dw],
                        ht_d[kd * (D // KCH):(kd + 1) * (D // KCH),
                             ct:ct + dw])
                    htts.append(htt)
                for half in range(0, dw, COLTILE):
                    nt = min(COLTILE, dw - half)
                    c0 = ct + half
                    leaf = c0 < NLEAFC
                    ps = ps_swp.tile([LC, COLTILE], F32, tag="ps_sw",
                                     name="ps_sw")
                    for kd in range(KCH):
                        nc.tensor.matmul(
                            ps[:, :nt], w_sb[:, kd, :],
                            htts[kd][:, half:half + nt],
                            start=(kd == 0), stop=(kd == KCH - 1))
                    if leaf:
                        # normalized already; deinterleave + bias into y9
                        pair0 = c0 // 2
                        nh = nt // 2
                        y9 = ybufs[DEPTH]
                        nc.vector.tensor_scalar(
                            y9[0:LC, pair0:pair0 + nh], ps[:, 0:nt:2],
                            b_sb[:, 0:1], None, ADD)
                        nc.vector.tensor_scalar(
                            y9[ROWR:ROWR + LC, pair0:pair0 + nh],
                            ps[:, 1:nt:2], b_sb[:, 0:1], None, ADD)
                    else:
                        nc.vector.tensor_scalar(
                            sw_sb[0:LC, c0 - NLEAFC:c0 - NLEAFC + nt],
                            ps[:, :nt], b_sb[:, 0:1], None, ADD)
                ct += dw

            # ---- phase 2: bottom-up tree levels ----
            for d in range(DEPTH - 1, -1, -1):
                n = TPC * (1 << d)
                yprev = ybufs[d + 1]
                p_off = OFFSI[d]
                # fold the completed child level's absolute state-0 scores
                # (ybuf rows 0 and 64) into the per-tree z accumulator
                npair_pt = (1 << d)          # pairs per tree in yprev
                for row in (0, ROWR):
                    rsum = utp.tile([1, TPC], F32, tag="rsum", name="rsum")
                    nc.vector.tensor_reduce(
                        rsum[:], yprev[row:row + 1, :].rearrange(
                            "p (t q) -> p t q", t=TPC),
                        mybir.AxisListType.X, ADD)
                    nc.vector.tensor_add(zacc[:], zacc[:], rsum[:])
                for c0 in range(0, n, COLTILE):
                    nt = min(COLTILE, n - c0)
                    ops_ = ps_outp.tile([LC, COLTILE], F32, tag="ps_out",
                                        name="ps_out")
                    for kp in range(KCH // 2):
                        eps = ps_expp.tile([CHW, 2, COLTILE], F32,
                                           tag="ps_exp", name="ps_exp")
                        for kk in range(2):
                            kc = 2 * kp + kk
                            nc.tensor.matmul(
                                eps[:, kk, :nt],
                                sellr_sb[:, kc * CHW:(kc + 1) * CHW],
                                yprev[0:NROWY, c0:c0 + nt],
                                start=True, stop=True)
                        v_sb = vtp.tile([CHW, 2, COLTILE], BF16, tag="v",
                                        name="v")
                        nc.scalar.activation(v_sb[:, :, :nt],
                                             eps[:, :, :nt], EXP)
                        for kk in range(2):
                            kc = 2 * kp + kk
                            nc.tensor.matmul(
                                ops_[:, :nt], expt_sb[:, kc, :],
                                v_sb[:, kk, :nt],
                                start=(kc == 0), stop=(kc == KCH - 1))

                    tdt = F32 if d == 0 else BF16
                    t_sb = ttp.tile([LC, COLTILE], tdt, tag=f"t{tdt}",
                                    name="t")
                    nc.scalar.activation(t_sb[:, :nt], ops_[:, :nt], LN)

                    if d == 0:
                        # final: Y = t + p_norm + (ztotal + p0_root) bcast,
                        # then undo the double-counted p0 on row 0.
                        nc.vector.tensor_add(
                            zfin[:], zacc[:],
                            sw_sb[0:1, p_off:p_off + nt])
                        qps = ps_normp.tile([LC, COLTILE], F32,
                                            tag="ps_norm", name="ps_norm")
                        nc.tensor.matmul(qps[:, :nt], onesr_sb[:],
                                         zfin[:], start=True, stop=True)
                        y0a = utp.tile([LC, TPC], F32, tag="y0a", name="y0a")
                        nc.vector.tensor_add(
                            y0a[:], t_sb[:, :nt],
                            sw_sb[0:LC, p_off:p_off + nt])
                        y0b = utp.tile([LC, TPC], F32, tag="y0b", name="y0b")
                        nc.vector.tensor_add(y0b[:], y0a[:], qps[:, :nt])
                        nc.vector.tensor_sub(
                            y0b[0:1, :], y0b[0:1, :],
                            sw_sb[0:1, p_off:p_off + nt])
                        nc.sync.dma_start(out_d[:], y0b[:])
                        continue

                    # normalize ln-scores, add normalized p, deinterleave
                    pn = ps_normp.tile([LC, COLTILE], F32, tag="ps_norm",
                                       name="ps_norm")
                    nc.tensor.matmul(pn[:, :nt], normmat_sb[:],
                                     t_sb[:, :nt], start=True, stop=True)
                    pair0 = c0 // 2
                    nh = nt // 2
                    yb = ybufs[d]
                    nc.vector.tensor_add(
                        yb[0:LC, pair0:pair0 + nh], pn[:, 0:nt:2],
                        sw_sb[0:LC, p_off + c0:p_off + c0 + nt:2])
                    nc.vector.tensor_add(
                        yb[ROWR:ROWR + LC, pair0:pair0 + nh], pn[:, 1:nt:2],
                        sw_sb[0:LC, p_off + c0 + 1:p_off + c0 + nt:2])

    nc.compile()
    _patch_act_tables(nc)
    return nc


_CACHE = {}


def _get_nc():
    if "nc" not in _CACHE:
        _CACHE["nc"] = _build_bass()
    return _CACHE["nc"]


def run(h, W, b, trans, trace=False, **trace_kwargs):
    h = np.asarray(h, dtype=np.float32)
    W = np.asarray(W, dtype=np.float32)
    b = np.asarray(b, dtype=np.float32)
    trans = np.asarray(trans, dtype=np.float32)

    consts = _host_constants(W, b, trans)
    in_maps = []
    for core in range(NCORES):
        m = dict(consts)
        m["ht"] = _host_ht(h, core)
        in_maps.append(m)

    nc = _get_nc()
    res = run_bass_kernel_spmd(nc, in_maps, list(range(NCORES)),
                               trace=trace, **trace_kwargs)
    outs = [res.results[k]["out"] for k in range(NCORES)]  # each [20, 8]
    full = np.concatenate([np.asarray(o, np.float32).T for o in outs],
                          axis=0).reshape(B, L, C)
    return np.ascontiguousarray(full), res


def kernel(h, W, b, trans):
    out, _ = run(h, W, b, trans, trace=False)
    return out


# revision 18
# speedup vs baseline: 1.1011x; 1.0735x over previous
"""Trainium2 Bass kernel for BinaryTreeLatentVariable inside algorithm.

Math (per level d, bottom-up over a complete binary tree in heap order):
    new[pp, n] = p[pp, n] + logsumexp_{i,j}( trans[pp, i, j] + l[i, n] + r[j, n] )

Factorization (s[n] = l[0, n] + r[0, n]):
    new[pp, n] = p[pp, n] + s[n] + log( sum_{ij} expT[ij, pp] * V[ij, n] )
    V[ij, n]   = exp( lnorm[i, n] + rnorm[j, n] ),  xnorm[i] = x[i] - x[0]
    expT       = exp(trans) permuted to [(lL,lc),(rL,rc)] x [(pL,pc)]

Representation: levels are stored NORMALIZED (relative to each node's
state-0 score, so values stay within ~+-12 and bf16 matmuls are safe) and
DEINTERLEAVED (sibling pairs share a column: left child in partitions
0..19, right child in partitions 64..83; state-0 rows carry don't-care
absolute values — the select matrices have zero weight there because
xnorm[0] == 0).  Absolute state-0 scores flow through a separate fp32
z-chain: zsum[n] = l[0,n] + r[0,n] per parent node.

Per level tile (nodes on the free axis):
    - 4x select matmul (K=84, bf16 0/1 matrix): args = lnorm_i + rnorm_j,
      written into paired PSUM banks so one ACT exp covers two chunks
    - 2x ACT exp (PSUM pair -> SBUF bf16), 4x contraction matmul with expT
    - ACT ln of the accumulated sums, DVE u = ln + p, normalization matmul
      (K=20, columns e_i - e_0; column 0 = e_0 keeps the absolute row),
      DVE deinterleave into the next ybuf
    - GPSIMD carries the fp32 z-chain (zrow = ln0 + (p0 + zsum); pairs)

Phase 1: emission sw = W^T @ hT + b on PE; h is cast to bf16 host-side
(halving HBM traffic) and laid out level-major (leaves first) so the
deepest level overlaps the tail of the h DMA.  Leaf columns use
host-normalized weights Wn (column i -> W_i - W_0, except column 0) so
leaf outputs drop straight into the normalized representation.

Sharding: 8 trees per core across 8 cores (no cross-core communication).
"""

import ml_dtypes
import numpy as np

import concourse.bacc as bacc
import concourse.bass as bass
from concourse import mybir, tile
from concourse.bass_utils import run_bass_kernel_spmd

F32 = mybir.dt.float32
BF16 = mybir.dt.bfloat16
NP_BF16 = ml_dtypes.bfloat16

B = 64
N_NODES = 1023
D = 512
L = 5
C = 4
LC = L * C          # 20
IJ = 400            # 20 * 20
NCORES = 8
TPC = B // NCORES   # trees per core = 8
DEPTH = 9           # leaves are level 9; internal levels 8..0

# Per-core column layout: level-major blocks (leaves first), t-major inside.
LEVEL_ORDER = list(range(DEPTH, -1, -1))  # 9, 8, ..., 0
OFFS = {}
_off = 0
for _d in LEVEL_ORDER:
    OFFS[_d] = _off
    _off += TPC * (1 << _d)
NCOL = _off                      # 8184
NLEAFC = TPC * (1 << DEPTH)      # 4096 leaf columns
NCOLI = NCOL - NLEAFC            # 4088 internal columns
OFFSI = {d: OFFS[d] - NLEAFC for d in range(DEPTH)}

COLTILE = 512
DMATILE = 2048
ROWR = 64           # partition base of the right-child (odd) block
NROWY = 84          # ybuf partitions: 0..19 left, 64..83 right, rest zero
KCH = 4             # 400 = 4 x 100 chunks of the ij axis
CHW = IJ // KCH     # 100


def _host_constants(W, b, trans):
    # expT: [400, 20], row = (lL*4+lc)*20 + (rL*4+rc), col = pL*4+pc,
    # chunked to [100, 4, 20] so SBUF tiles slice on a free dim.
    expT = np.exp(trans.astype(np.float64).transpose(1, 4, 2, 5, 0, 3)
                  .reshape(IJ, LC))
    expT_ch = np.ascontiguousarray(
        expT.reshape(KCH, CHW, LC).transpose(1, 0, 2)).astype(NP_BF16)

    ij = np.arange(IJ)
    selLR = np.zeros((NROWY, IJ), NP_BF16)
    selLR[ij // LC, ij] = 1.0            # left-child state select
    selLR[ROWR + ij % LC, ij] = 1.0      # right-child state select
    selLR[0, :] = 0.0                    # lnorm[0] == 0: ignore row 0
    selLR[ROWR, :] = 0.0                 # rnorm[0] == 0: ignore row 64

    # normalized weights everywhere: col i -> W_i - W_0 for i>0; col 0
    # keeps W_0 so row 0 of every emission is the absolute state-0 score.
    Wn = W - W[:, 0:1]
    Wn[:, 0] = W[:, 0]
    w_both = np.ascontiguousarray(
        Wn.reshape(KCH, D // KCH, LC)).astype(NP_BF16)

    bn = (b - b[0]).astype(np.float32)
    bn[0] = b[0]
    bcols = bn.reshape(LC, 1)

    # normalization: col i>0 -> u_i - u_0; col 0 -> u_0 (absolute kept)
    normmat = np.zeros((LC, LC), NP_BF16)
    normmat[0, 0] = 1.0
    for i in range(1, LC):
        normmat[i, i] = 1.0
        normmat[0, i] = -1.0
    ones_row = np.ones((1, LC), np.float32)
    return {
        "expt": expT_ch, "sellr": selLR, "wboth": w_both, "bcols": bcols,
        "normmat": normmat, "onesr": ones_row,
    }


def _host_ht(h, core):
    """bf16 [512, NCOL] slice for one core: level-major, t-major inside."""
    hk = h[core * TPC:(core + 1) * TPC]          # [8, 1023, 512]
    blocks = []
    for d in LEVEL_ORDER:
        lo, hi = (1 << d) - 1, (1 << (d + 1)) - 1
        blk = hk[:, lo:hi, :]                     # [8, m, 512]
        blocks.append(blk.transpose(2, 0, 1).reshape(D, -1))
    out = np.concatenate(blocks, axis=1)
    return np.ascontiguousarray(out).astype(NP_BF16)


def _patch_act_tables(nc):
    """Retarget every activation-table load to natural_log_exp_and_others
    (covers Exp, Ln and Identity) and drop the now-redundant reloads, which
    otherwise cost ~1.3us each when Exp and Ln alternate."""
    from concourse.hw_specs import get_activation_tables
    tables = list(get_activation_tables(nc.m.arch).items())
    target = None
    for idx, (name, _fns) in enumerate(tables):
        if name == "natural_log_exp_and_others":
            target = idx
    if target is None:
        return
    for fn in nc.m.functions:
        kept = False
        for blk in fn.blocks:
            new_insts = []
            for ins in blk.instructions:
                if isinstance(ins, mybir.InstLoadActFuncSet):
                    si = ins.sync_info
                    has_sems = si is not None and (
                        len(si.on_wait) > 0 or len(si.on_update) > 0)
                    if not kept or has_sems:
                        ins.act_func_set_id = target
                        kept = True
                        new_insts.append(ins)
                    continue
                new_insts.append(ins)
            blk.instructions[:] = new_insts


def _build_bass():
    nc = bacc.Bacc("TRN2", target_bir_lowering=False)

    ht_d = nc.declare_dram_parameter("ht", [D, NCOL], BF16, isOutput=False)
    wboth_d = nc.declare_dram_parameter("wboth", [KCH, D // KCH, LC],
                                        BF16, isOutput=False)
    bcols_d = nc.declare_dram_parameter("bcols", [LC, 1], F32, isOutput=False)
    expt_d = nc.declare_dram_parameter("expt", [CHW, KCH, LC], BF16,
                                       isOutput=False)
    sellr_d = nc.declare_dram_parameter("sellr", [NROWY, IJ], BF16,
                                        isOutput=False)
    normmat_d = nc.declare_dram_parameter("normmat", [LC, LC], BF16,
                                          isOutput=False)
    onesr_d = nc.declare_dram_parameter("onesr", [1, LC], F32, isOutput=False)
    out_d = nc.declare_dram_parameter("out", [LC, TPC], F32, isOutput=True)

    EXP = mybir.ActivationFunctionType.Exp
    LN = mybir.ActivationFunctionType.Ln
    ADD = mybir.AluOpType.add

    with tile.TileContext(nc) as tc:
        with (
            tc.tile_pool(name="consts", bufs=1) as consts,
            tc.tile_pool(name="sw", bufs=1) as swp,
            tc.tile_pool(name="ybufs", bufs=1) as ybp,
            tc.tile_pool(name="ht", bufs=8) as htp,
            tc.tile_pool(name="vtiles", bufs=4) as vtp,
            tc.tile_pool(name="utiles", bufs=3) as utp,
            tc.tile_pool(name="ttiles", bufs=2) as ttp,
            tc.tile_pool(name="ps_sw", bufs=2, space="PSUM") as ps_swp,
            tc.tile_pool(name="ps_exp", bufs=2, space="PSUM") as ps_expp,
            tc.tile_pool(name="ps_out", bufs=1, space="PSUM") as ps_outp,
            tc.tile_pool(name="ps_norm", bufs=1, space="PSUM") as ps_normp,
        ):
            # ---- constants (issued on the GPSIMD queue so the SP queue
            # starts streaming hT immediately) ----
            w_sb = consts.tile([D // KCH, KCH, LC], BF16)
            nc.gpsimd.dma_start(w_sb[:], wboth_d[:].transpose([1, 0, 2]))
            expt_sb = consts.tile([CHW, KCH, LC], BF16)
            nc.gpsimd.dma_start(expt_sb[:], expt_d[:])
            sellr_sb = consts.tile([NROWY, IJ], BF16)
            nc.gpsimd.dma_start(sellr_sb[:], sellr_d[:])
            b_sb = consts.tile([LC, 1], F32)
            nc.gpsimd.dma_start(b_sb[:], bcols_d[:])
            normmat_sb = consts.tile([LC, LC], BF16)
            nc.gpsimd.dma_start(normmat_sb[:], normmat_d[:])
            onesr_sb = consts.tile([1, LC], F32)
            nc.gpsimd.dma_start(onesr_sb[:], onesr_d[:])

            # normalized emissions of internal nodes (row 0 = absolute p0)
            sw_sb = swp.tile([LC, NCOLI], F32)
            # per-tree accumulator of absolute state-0 scores (z-total)
            zacc = swp.tile([1, TPC], F32)
            zfin = swp.tile([1, TPC], F32)
            nc.vector.memset(zacc[:], 0.0)

            # per-level normalized deinterleaved buffers, bf16
            ybufs = {}
            for d in range(DEPTH, 0, -1):
                npair = TPC * (1 << d) // 2
                yb = ybp.tile([NROWY, npair], BF16, tag=f"y{d}", name=f"y{d}")
                nc.gpsimd.memset(yb[:], 0.0)
                ybufs[d] = yb

            # ---- phase 1: sw / leaf level from hT ----
            for ct in range(0, NCOL, DMATILE):
                dw = min(DMATILE, NCOL - ct)
                htts = []
                for kd in range(KCH):
                    htt = htp.tile([D // KCH, DMATILE], BF16, tag="htt",
                                   name="htt")
                    nc.sync.dma_start(
                        htt[:, :dw],
                        ht_d[kd * (D // KCH):(kd + 1) * (D // KCH),
                             ct:ct + dw])
                    htts.append(htt)
                for half in range(0, dw, COLTILE):
                    nt = min(COLTILE, dw - half)
                    c0 = ct + half
                    leaf = c0 < NLEAFC
                    ps = ps_swp.tile([LC, COLTILE], F32, tag="ps_sw",
                                     name="ps_sw")
                    for kd in range(KCH):
                        nc.tensor.matmul(
                            ps[:, :nt], w_sb[:, kd, :],
                            htts[kd][:, half:half + nt],
                            start=(kd == 0), stop=(kd == KCH - 1))
                    if leaf:
                        # normalized already; deinterleave + bias into y9
                        pair0 = c0 // 2
                        nh = nt // 2
                        y9 = ybufs[DEPTH]
                        nc.vector.tensor_scalar(
                            y9[0:LC, pair0:pair0 + nh], ps[:, 0:nt:2],
                            b_sb[:, 0:1], None, ADD)
                        nc.vector.tensor_scalar(
                            y9[ROWR:ROWR + LC, pair0:pair0 + nh],
                            ps[:, 1:nt:2], b_sb[:, 0:1], None, ADD)
                    else:
                        nc.vector.tensor_scalar(
                            sw_sb[0:LC, c0 - NLEAFC:c0 - NLEAFC + nt],
                            ps[:, :nt], b_sb[:, 0:1], None, ADD)

            # ---- phase 2: bottom-up tree levels ----
            for d in range(DEPTH - 1, -1, -1):
                n = TPC * (1 << d)
                yprev = ybufs[d + 1]
                p_off = OFFSI[d]
                # fold the completed child level's absolute state-0 scores
                # (ybuf rows 0 and 64) into the per-tree z accumulator
                npair_pt = (1 << d)          # pairs per tree in yprev
                for row in (0, ROWR):
                    rsum = utp.tile([1, TPC], F32, tag="rsum", name="rsum")
                    nc.vector.tensor_reduce(
                        rsum[:], yprev[row:row + 1, :].rearrange(
                            "p (t q) -> p t q", t=TPC),
                        mybir.AxisListType.X, ADD)
                    nc.vector.tensor_add(zacc[:], zacc[:], rsum[:])
                for c0 in range(0, n, COLTILE):
                    nt = min(COLTILE, n - c0)
                    ops_ = ps_outp.tile([LC, COLTILE], F32, tag="ps_out",
                                        name="ps_out")
                    for kp in range(KCH // 2):
                        eps = ps_expp.tile([CHW, 2, COLTILE], F32,
                                           tag="ps_exp", name="ps_exp")
                        for kk in range(2):
                            kc = 2 * kp + kk
                            nc.tensor.matmul(
                                eps[:, kk, :nt],
                                sellr_sb[:, kc * CHW:(kc + 1) * CHW],
                                yprev[0:NROWY, c0:c0 + nt],
                                start=True, stop=True)
                        v_sb = vtp.tile([CHW, 2, COLTILE], BF16, tag="v",
                                        name="v")
                        nc.scalar.activation(v_sb[:, :, :nt],
                                             eps[:, :, :nt], EXP)
                        for kk in range(2):
                            kc = 2 * kp + kk
                            nc.tensor.matmul(
                                ops_[:, :nt], expt_sb[:, kc, :],
                                v_sb[:, kk, :nt],
                                start=(kc == 0), stop=(kc == KCH - 1))

                    tdt = F32 if d == 0 else BF16
                    t_sb = ttp.tile([LC, COLTILE], tdt, tag=f"t{tdt}",
                                    name="t")
                    nc.scalar.activation(t_sb[:, :nt], ops_[:, :nt], LN)

                    if d == 0:
                        # final: Y = t + p_norm + (ztotal + p0_root) bcast,
                        # then undo the double-counted p0 on row 0.
                        nc.vector.tensor_add(
                            zfin[:], zacc[:],
                            sw_sb[0:1, p_off:p_off + nt])
                        qps = ps_normp.tile([LC, COLTILE], F32,
                                            tag="ps_norm", name="ps_norm")
                        nc.tensor.matmul(qps[:, :nt], onesr_sb[:],
                                         zfin[:], start=True, stop=True)
                        y0a = utp.tile([LC, TPC], F32, tag="y0a", name="y0a")
                        nc.vector.tensor_add(
                            y0a[:], t_sb[:, :nt],
                            sw_sb[0:LC, p_off:p_off + nt])
                        y0b = utp.tile([LC, TPC], F32, tag="y0b", name="y0b")
                        nc.vector.tensor_add(y0b[:], y0a[:], qps[:, :nt])
                        nc.vector.tensor_sub(
                            y0b[0:1, :], y0b[0:1, :],
                            sw_sb[0:1, p_off:p_off + nt])
                        nc.sync.dma_start(out_d[:], y0b[:])
                        continue

                    # normalize ln-scores, add normalized p, deinterleave
                    pn = ps_normp.tile([LC, COLTILE], F32, tag="ps_norm",
                                       name="ps_norm")
                    nc.tensor.matmul(pn[:, :nt], normmat_sb[:],
                                     t_sb[:, :nt], start=True, stop=True)
                    pair0 = c0 // 2
                    nh = nt // 2
                    yb = ybufs[d]
                    nc.vector.tensor_add(
                        yb[0:LC, pair0:pair0 + nh], pn[:, 0:nt:2],
                        sw_sb[0:LC, p_off + c0:p_off + c0 + nt:2])
                    nc.vector.tensor_add(
                        yb[ROWR:ROWR + LC, pair0:pair0 + nh], pn[:, 1:nt:2],
                        sw_sb[0:LC, p_off + c0 + 1:p_off + c0 + nt:2])

    nc.compile()
    _patch_act_tables(nc)
    return nc


_CACHE = {}


def _get_nc():
    if "nc" not in _CACHE:
        _CACHE["nc"] = _build_bass()
    return _CACHE["nc"]


def run(h, W, b, trans, trace=False, **trace_kwargs):
    h = np.asarray(h, dtype=np.float32)
    W = np.asarray(W, dtype=np.float32)
    b = np.asarray(b, dtype=np.float32)
    trans = np.asarray(trans, dtype=np.float32)

    consts = _host_constants(W, b, trans)
    in_maps = []
    for core in range(NCORES):
        m = dict(consts)
        m["ht"] = _host_ht(h, core)
        in_maps.append(m)

    nc = _get_nc()
    res = run_bass_kernel_spmd(nc, in_maps, list(range(NCORES)),
                               trace=trace, **trace_kwargs)
    outs = [res.results[k]["out"] for k in range(NCORES)]  # each [20, 8]
    full = np.concatenate([np.asarray(o, np.float32).T for o in outs],
                          axis=0).reshape(B, L, C)
    return np.ascontiguousarray(full), res


def kernel(h, W, b, trans):
    out, _ = run(h, W, b, trans, trace=False)
    return out
